# revision 19
# baseline (speedup 1.0000x reference)
"""RWKV-style CausalEventModel kernel for 8 Trainium2 NeuronCores.

Strategy (zero cross-core communication):
  - Data-parallel over batch (B=4) x 2-way sequence split per batch = 8 cores.
  - Each core runs the FULL model on M=1088 tokens in channel-major layout
    ([D partitions, tokens free]).  The second-half core starts W=128 tokens
    early with zero initial WKV state; the per-channel decay makes the
    missing-prefix contribution negligible by the output region.
  - Two token half-blocks (512 / 576) are software-pipelined through every
    layer phase; the WKV recurrence state chains across halves via the scan's
    `initial` operand.

V2 performance rework (vs. the first working version):
  - One ACT table-set discipline: Exp/Tanh/Square/Relu/Copy/Identity all live
    in exp_and_others; only the LN-row Sqrt swaps to sqrt_and_others (2 swaps
    per phase, stats for both halves batched at phase start).
  - sigmoid(x) = 0.5*(tanh(x/2)+1): computed as Tanh on the ACT engine with
    the 0.5 folded into Wv / Wcv host-side, so no sigmoid table set is needed.
  - WKV assembly fused into scalar_tensor_tensor with e^u as the per-channel
    scalar: num = ekv*e^u + A_shift, den = ek*e^u + B_shift.  1/den uses the
    single-instruction DVE reciprocal_approx_fast (fp32, ~18 bits); the LN row
    reciprocal uses it too.
  - mix outputs computed as z + (mix-1)*d with the host packing (mix-1), so
    both STT tensor operands read at aligned even offsets; z/sc/scB tiles are
    [P, 2+M] keeping the partition pitch 4B-aligned for DVE packed modes.
  - Channel-mix relu()^2: ACT Relu epilogue + one in-place 4-wide DVE
    tensor_tensor square per group at bf16 2x rate.
  - HAM keep-alive: tiny ones-matmuls hang off just-produced elementwise
    results inside the scan/apply windows so the PE clock stays at 2.4 GHz.
  - 3-deep "mm" PSUM rotation + double-buffered wk/wv DMAs; k-mix tiles are
    emitted first so the k-projection starts as early as possible.
"""
import numpy as np
import ml_dtypes

B, T, E, D, F, L, OUT = 4, 2048, 4, 512, 2048, 8, 512
P = 128
KD = D // P          # 4
KF = F // P          # 16
W_WARM = 128
M = (T + W_WARM) // 2        # 1088 tokens per core
S_SPLIT = M                  # first-half output rows
HALVES = [(0, 512), (512, M - 512)]          # token half-blocks per core
N_CORES = 8
EPS = 1e-5

_CACHE = {}


def _mm_slices(cn):
    """Output-column slices (relative to a PSUM tile start) that keep each
    matmul's output inside one 2KB PSUM bank."""
    out = [(0, min(512, cn))]
    if cn > 512:
        out.append((512, cn - 512))
    return out


def _build_bass():
    import concourse.bass as bass
    import concourse.bacc as bacc
    import concourse.mybir as mybir
    import concourse.tile as tile
    from contextlib import ExitStack

    f32 = mybir.dt.float32
    f32r = mybir.dt.float32r
    b16 = mybir.dt.bfloat16
    AF = mybir.ActivationFunctionType
    OP = mybir.AluOpType

    nc = bacc.Bacc("TRN2", target_bir_lowering=False, debug=False)

    # ---------------- DRAM tensors ----------------
    xT_d = nc.dram_tensor("xT", [P, M], b16, kind="ExternalInput")
    wemb_d = nc.dram_tensor("wemb", [P, D], b16, kind="ExternalInput")
    inv_d = nc.dram_tensor("inv", [P, 12], f32, kind="ExternalInput")
    wk_d = nc.dram_tensor("wk", [L, KD, P, D], b16, kind="ExternalInput")
    wv_d = nc.dram_tensor("wv", [L, KD, P, D], b16, kind="ExternalInput")
    wr_d = nc.dram_tensor("wr", [L, KD, P, D], b16, kind="ExternalInput")
    wo_d = nc.dram_tensor("wo", [L, KD, P, D], b16, kind="ExternalInput")
    wck_d = nc.dram_tensor("wck", [L, KD, P, F], b16, kind="ExternalInput")
    wcv_d = nc.dram_tensor("wcv", [L, KF, P, D], b16, kind="ExternalInput")
    wcr_d = nc.dram_tensor("wcr", [L, KD, P, D], b16, kind="ExternalInput")
    whead_d = nc.dram_tensor("whead", [KD, P, OUT], b16, kind="ExternalInput")
    tmv_d = nc.dram_tensor("tmv", [L, P, 32], f32, kind="ExternalInput")
    cmv_d = nc.dram_tensor("cmv", [L, P, 28], f32, kind="ExternalInput")
    headb_d = nc.dram_tensor("headb", [P, KD], f32, kind="ExternalInput")
    out_d = nc.dram_tensor("outT", [KD, P, M], f32, kind="ExternalOutput")

    ctx = ExitStack()
    tc = ctx.enter_context(tile.TileContext(nc))
    sb = ctx.enter_context(tc.tile_pool(name="sb", bufs=1))
    vp = ctx.enter_context(tc.tile_pool(name="vp", bufs=2))
    wp = ctx.enter_context(tc.tile_pool(name="wp", bufs=1))
    pp = ctx.enter_context(tc.tile_pool(name="pp", bufs=2, space="PSUM"))

    # persistent tiles
    h = [sb.tile([P, M], f32, name=f"h{j}", tag=f"h{j}") for j in range(KD)]
    ones_b = sb.tile([P, 1], b16, name="ones_b", tag="ones_b")
    nc.vector.memset(ones_b, 1.0)
    ones_f = sb.tile([P, 1], f32, name="ones_f", tag="ones_f")
    nc.vector.memset(ones_f, 1.0)
    ones_row = sb.tile([1, P], b16, name="ones_row", tag="ones_row")
    nc.vector.memset(ones_row, 1.0)
    srowA = sb.tile([1, M], f32, name="srowA", tag="srowA")
    srowB = sb.tile([1, M], f32, name="srowB", tag="srowB")
    rb0 = sb.tile([1, M], b16, name="rb0", tag="rb0")
    rb1 = sb.tile([1, M], b16, name="rb1", tag="rb1")
    eps_col = sb.tile([P, 1], f32, name="eps_col", tag="eps_col")
    nc.vector.memset(eps_col, EPS)
    rstd_sb = sb.tile([P, M], b16, name="rstd_sb", tag="rstd_sb")
    mean_sb = sb.tile([P, M], b16, name="mean_sb", tag="mean_sb")

    def alloc4(prefix, width, dtype, pool=sb, tagp=None):
        tagp = tagp or prefix
        return [pool.tile([P, width], dtype, name=f"{prefix}{j}", tag=f"{tagp}{j}")
                for j in range(KD)]

    def ln_stats(h_tiles, sq_tiles, z_tiles, c0, cn):
        """Per-token mean/rstd of h[:, c0:c0+cn] over 512 channels into
        rstd_sb / mean_sb (bf16 broadcast tiles, absolute token columns).
        Leaves a bf16 copy of h in z[:, 2+c0:] for the sum matmul; LN-apply
        then normalizes z in place.  z tiles are [P, 2+M] so the partition
        pitch stays 4B-aligned (DVE 2x packed mode eligibility)."""
        cs = slice(c0, c0 + cn)
        zs = slice(2 + c0, 2 + c0 + cn)
        for j in range(KD):
            nc.vector.tensor_copy(out=z_tiles[j][:, zs], in_=h_tiles[j][:, cs])
        # sq = h^2 in bf16 (Square is in every ACT table set)
        for j in range(KD):
            nc.scalar.activation(out=sq_tiles[j][:, cs], in_=h_tiles[j][:, cs],
                                 func=AF.Square)
        for (s0, sn) in _mm_slices(cn):
            a0 = c0 + s0
            sl = slice(a0, a0 + sn)
            sum_ps = pp.tile([1, 512], f32, name="sum_ps", tag="st")
            sq_ps = pp.tile([1, 512], f32, name="sq_ps", tag="st")
            for j in range(KD):
                nc.tensor.matmul(sum_ps[0:1, :sn],
                                 lhsT=ones_b,
                                 rhs=z_tiles[j][:, 2 + a0:2 + a0 + sn],
                                 start=(j == 0), stop=(j == KD - 1))
            for j in range(KD):
                nc.tensor.matmul(sq_ps[0:1, :sn], lhsT=ones_b,
                                 rhs=sq_tiles[j][:, sl],
                                 start=(j == 0), stop=(j == KD - 1))
            # row math on partition 0: var = sqsum/D - (sum/D)^2
            sB = srowB[0:1, sl]
            nc.scalar.activation(out=sB, in_=sum_ps[0:1, :sn], func=AF.Square,
                                 scale=1.0 / D)
            nc.vector.scalar_tensor_tensor(out=srowA[0:1, sl],
                                           in0=sq_ps[0:1, :sn],
                                           scalar=1.0 / D, in1=sB,
                                           op0=OP.mult, op1=OP.subtract)
            nc.scalar.activation(out=srowA[0:1, sl], in_=srowA[0:1, sl],
                                 func=AF.Sqrt, bias=eps_col[0:1, :])
            nc.vector.reciprocal_approx_fast(out=srowB[0:1, sl],
                                             in_=srowA[0:1, sl])
            with nc.allow_low_precision(reason="per-token rstd in bf16"):
                nc.vector.tensor_copy(out=rb0[0:1, sl], in_=srowB[0:1, sl])
                nc.scalar.activation(out=rb1[0:1, sl], in_=sum_ps[0:1, :sn],
                                     func=AF.Copy, scale=1.0 / D)
            # broadcast across partitions: K=1 matmul -> PSUM -> bf16 SBUF
            bc_ps = pp.tile([P, 576], f32, name="bc_ps", tag="mm", bufs=3)
            bc_ps2 = pp.tile([P, 576], f32, name="bc_ps2", tag="mm", bufs=3)
            nc.tensor.matmul(bc_ps[:, :sn], lhsT=ones_row, rhs=rb0[0:1, sl],
                             start=True, stop=True)
            nc.tensor.matmul(bc_ps2[:, :sn], lhsT=ones_row, rhs=rb1[0:1, sl],
                             start=True, stop=True)
            nc.scalar.activation(out=rstd_sb[:, sl], in_=bc_ps[:, :sn],
                                 func=AF.Copy)
            nc.scalar.activation(out=mean_sb[:, sl], in_=bc_ps2[:, :sn],
                                 func=AF.Copy)

    def ka(src_tile, c0):
        """HAM keep-alive: a tiny matmul that depends on a just-produced
        elementwise result, so the PE sees activity inside long vector-only
        windows and its clock stays at K=8/8 (2.4 GHz)."""
        ka_ps = pp.tile([1, 512], f32, name="ka_ps", tag="st")
        nc.tensor.matmul(ka_ps[0:1, :64], lhsT=ones_b,
                         rhs=src_tile[:, c0:c0 + 64],
                         start=True, stop=True)

    def ln_apply_z(z_tiles, c0, cn):
        """z = (z - mean)*rstd in place on the half-block (GpSimd)."""
        cs = slice(c0, c0 + cn)
        zs = slice(2 + c0, 2 + c0 + cn)
        for j in range(KD):
            eng = nc.vector if j % 2 == 0 else nc.gpsimd
            eng.tensor_tensor(z_tiles[j][:, zs], z_tiles[j][:, zs],
                              mean_sb[:, cs], OP.subtract)
        for j in range(KD):
            eng = nc.vector if j % 2 == 0 else nc.gpsimd
            eng.tensor_tensor(z_tiles[j][:, zs], z_tiles[j][:, zs],
                              rstd_sb[:, cs], OP.mult)
            if j % 2 == 1:
                ka(z_tiles[j], 2 + c0)

    def mixes(z_t, d_t, outs_scalars, vec_t, c0, cn, kas=False):
        """out = mix*z + (1-mix)*z_sh = z + (mix-1)*d with d = z - z_sh.
        The host packs (mix-1) so both STT tensor operands read at aligned
        even offsets -> DVE 2x.  d lives in the dead sq tiles."""
        for j in range(KD):
            zc = z_t[j][:, 2 + c0:2 + c0 + cn]
            zsh = z_t[j][:, 1 + c0:1 + c0 + cn]
            eng = nc.vector if j % 2 == 0 else nc.gpsimd
            eng.tensor_tensor(d_t[j][:, c0:c0 + cn], zc, zsh, OP.subtract)
            if kas and j % 2 == 1:
                ka(d_t[j], c0)
        for (out_tiles, col) in outs_scalars:
            for j in range(KD):
                zc = z_t[j][:, 2 + c0:2 + c0 + cn]
                nc.vector.scalar_tensor_tensor(out=out_tiles[j][:, c0:c0 + cn],
                                               in0=d_t[j][:, c0:c0 + cn],
                                               scalar=vec_t[:, col + j:col + j + 1],
                                               in1=zc, op0=OP.mult, op1=OP.add)

    def proj(rhs_tiles, w_t, c0, cn, epilogue):
        """epilogue(m, ps) consumes the [P, cn] PSUM of output tile m."""
        for m in range(KD):
            ps = pp.tile([P, 576], f32, name="proj_ps", tag="mm", bufs=3)
            for (s0, sn) in _mm_slices(cn):
                for kj in range(KD):
                    nc.tensor.matmul(
                        ps[:, s0:s0 + sn],
                        lhsT=w_t[:, kj, m * P:(m + 1) * P],
                        rhs=rhs_tiles[kj][:, c0 + s0:c0 + s0 + sn],
                        start=(kj == 0), stop=(kj == KD - 1))
            epilogue(m, ps)

    # ---------------- embedding ----------------
    xt = sb.tile([P, M], b16, name="xt", tag="xt")
    nc.gpsimd.dma_start(out=xt, in_=xT_d[:, :])
    wemb_t = sb.tile([P, D], b16, name="wemb_t", tag="wemb_t")
    nc.gpsimd.dma_start(out=wemb_t, in_=wemb_d[:, :])
    inv_t = sb.tile([P, 12], f32, name="inv_t", tag="inv_t")
    nc.gpsimd.dma_start(out=inv_t, in_=inv_d[:, :])
    headb_t = sb.tile([P, KD], f32, name="headb_t", tag="headb_t")
    nc.gpsimd.dma_start(out=headb_t, in_=headb_d[:, :])

    for (c0, cn) in HALVES:
        for m in range(KD):
            ps = pp.tile([P, 576], f32, name=f"emb_ps{m}", tag="mm", bufs=3)
            for (s0, sn) in _mm_slices(cn):
                nc.tensor.matmul(ps[:, s0:s0 + sn],
                                 lhsT=wemb_t[:, m * P:(m + 1) * P],
                                 rhs=xt[:, c0 + s0:c0 + s0 + sn],
                                 start=True, stop=True)
            nc.scalar.activation(out=h[m][:, c0:c0 + cn], in_=ps[:, :cn],
                                 func=AF.Identity, bias=inv_t[:, m:m + 1])

    # ln_in (explicit w/b application since h is the residual stream)
    sq = alloc4("sq", M, b16)
    z = [sb.tile([P, 2 + M], b16, name=f"z{j}", tag=f"z{j}") for j in range(KD)]
    for j in range(KD):
        nc.vector.memset(z[j][:, 1:2], 0.0)
    for (c0, cn) in HALVES:
        ln_stats(h, sq, z, c0, cn)
        ln_apply_z(z, c0, cn)
        for j in range(KD):
            nc.vector.tensor_scalar(out=h[j][:, c0:c0 + cn],
                                    in0=z[j][:, 2 + c0:2 + c0 + cn],
                                    scalar1=inv_t[:, 4 + j:5 + j],
                                    scalar2=inv_t[:, 8 + j:9 + j],
                                    op0=OP.mult, op1=OP.add)

    # ---------------- layers ----------------
    for li in range(L):
        tmv_t = vp.tile([P, 32], f32, name=f"tmv{li}", tag="tmv")
        nc.sync.dma_start(out=tmv_t, in_=tmv_d[li])
        wk_t = wp.tile([P, KD, D], b16, name=f"wk{li}", tag="wk", bufs=2)
        nc.sync.dma_start(out=wk_t, in_=wk_d[li].rearrange("k p d -> p k d"))
        wv_t = wp.tile([P, KD, D], b16, name=f"wv{li}", tag="wv", bufs=2)
        nc.sync.dma_start(out=wv_t, in_=wv_d[li].rearrange("k p d -> p k d"))
        wr_t = wp.tile([P, KD, D], b16, name=f"wr{li}", tag="wrc")
        nc.sync.dma_start(out=wr_t, in_=wr_d[li].rearrange("k p d -> p k d"))
        wo_t = wp.tile([P, KD, D], b16, name=f"wo{li}", tag="wock")
        nc.sync.dma_start(out=wo_t, in_=wo_d[li].rearrange("k p d -> p k d"))

        # ---- time mixing ----
        sq = alloc4("sq", M, b16)
        z = [sb.tile([P, 2 + M], b16, name=f"z{j}", tag=f"z{j}")
             for j in range(KD)]
        xkz = alloc4("xkz", M, b16)
        xvz = alloc4("xvz", M, b16)
        xrz = alloc4("xrz", M, b16)
        # ek/vv/rr/sc live in the channel-mix kf slots (disjoint lifetime)
        ek = alloc4("ek", M, b16)
        vv = alloc4("vv", M, b16)
        rr = alloc4("rr", M, b16)
        sc = [sb.tile([P, 2 + M], b16, name=f"sc{j}", tag=f"sc{j}")
              for j in range(KD)]
        scB = [sb.tile([P, 2 + M], b16, name=f"scB{j}", tag=f"t2_{j}")
               for j in range(KD)]
        den = [sb.tile([P, 576], f32, name=f"den{j}", tag=f"den{j}")
               for j in range(KD)]
        for j in range(KD):
            nc.vector.memset(z[j][:, 1:2], 0.0)
            nc.vector.memset(sc[j][:, 0:1], 0.0)
            nc.vector.memset(scB[j][:, 0:1], 0.0)

        # stats for both halves first: one Rsqrt table-set period per phase
        for (c0, cn) in HALVES:
            ln_stats(h, sq, z, c0, cn)

        def tm_mix_proj(c0, cn):
            ln_apply_z(z, c0, cn)
            mixes(z, sq, [(xkz, 0), (xvz, 4), (xrz, 8)], tmv_t, c0, cn,
                  kas=True)

            def k_epi(m, ps):
                nc.scalar.activation(out=ek[m][:, c0:c0 + cn], in_=ps[:, :cn],
                                     func=AF.Exp,
                                     bias=tmv_t[:, 20 + m:21 + m])

            def v_epi(m, ps):
                nc.scalar.activation(out=vv[m][:, c0:c0 + cn], in_=ps[:, :cn],
                                     func=AF.Identity,
                                     bias=tmv_t[:, 24 + m:25 + m])

            def r_epi(m, ps):
                nc.scalar.activation(out=rr[m][:, c0:c0 + cn], in_=ps[:, :cn],
                                     func=AF.Tanh, scale=0.5,
                                     bias=tmv_t[:, 28 + m:29 + m])

            proj(xkz, wk_t, c0, cn, k_epi)
            proj(xvz, wv_t, c0, cn, v_epi)
            proj(xrz, wr_t, c0, cn, r_epi)

        def tm_scans(c0, cn):
            cs = slice(c0, c0 + cn)
            for j in range(KD):
                # ekv into the xvz slot (v-proj has fully consumed it)
                nc.vector.tensor_tensor(xvz[j][:, cs], ek[j][:, cs],
                                        vv[j][:, cs], OP.mult)
            for j in range(KD):
                ew_ap = tmv_t[:, 12 + j:13 + j].to_broadcast([P, cn])
                init = 0.0 if c0 == 0 else sc[j][:, c0:c0 + 1]
                nc.vector.tensor_tensor_scan(out=sc[j][:, 1 + c0:1 + c0 + cn],
                                             data0=ew_ap, data1=xvz[j][:, cs],
                                             initial=init,
                                             op0=OP.mult, op1=OP.add)
                ka(sc[j], c0)
            for j in range(KD):
                ew_ap = tmv_t[:, 12 + j:13 + j].to_broadcast([P, cn])
                init = 0.0 if c0 == 0 else scB[j][:, c0:c0 + 1]
                nc.vector.tensor_tensor_scan(out=scB[j][:, 1 + c0:1 + c0 + cn],
                                             data0=ew_ap, data1=ek[j][:, cs],
                                             initial=init,
                                             op0=OP.mult, op1=OP.add)
                if j % 2 == 1:
                    ka(scB[j], c0)

        def tm_tail(c0, cn):
            cs = slice(c0, c0 + cn)
            for j in range(KD):
                # num = ekv*e^u + A_shift  (overwrite vv; all operands at
                # aligned even offsets -> DVE 2x)
                nc.vector.scalar_tensor_tensor(out=vv[j][:, cs],
                                               in0=xvz[j][:, cs],
                                               scalar=tmv_t[:, 16 + j:17 + j],
                                               in1=sc[j][:, c0:c0 + cn],
                                               op0=OP.mult, op1=OP.add)
                if j % 2 == 1:
                    ka(vv[j], c0)
            for j in range(KD):
                # den = ek*e^u + B_shift (fp32 for the fast reciprocal)
                nc.vector.scalar_tensor_tensor(out=den[j][:, :cn],
                                               in0=ek[j][:, cs],
                                               scalar=tmv_t[:, 16 + j:17 + j],
                                               in1=scB[j][:, c0:c0 + cn],
                                               op0=OP.mult, op1=OP.add)
            for j in range(KD):
                nc.vector.reciprocal_approx_fast(out=den[j][:, :cn],
                                                 in_=den[j][:, :cn])
            for j in range(KD):
                # rn = (tanh+1)*recip  (0.5 folded into Wv host-side)
                nc.vector.scalar_tensor_tensor(out=rr[j][:, cs],
                                               in0=rr[j][:, cs], scalar=1.0,
                                               in1=den[j][:, :cn],
                                               op0=OP.add, op1=OP.mult)
            for j in range(KD):
                # rwkv = num * (rr_scaled * recip) -> k-proj's dead xkz slot
                nc.vector.tensor_tensor(xkz[j][:, cs], vv[j][:, cs],
                                        rr[j][:, cs], OP.mult)

        def tm_out(c0, cn):
            cs = slice(c0, c0 + cn)

            def o_epi(m, ps):
                nc.vector.tensor_tensor(h[m][:, cs], h[m][:, cs], ps[:, :cn],
                                        OP.add)

            proj(xkz, wo_t, c0, cn, o_epi)

        tm_mix_proj(*HALVES[0])
        tm_mix_proj(*HALVES[1])

        # ---- channel-mix setup, hoisted so the cm h0 pipeline interleaves
        #      with the tm h1 scan/tail (PE keeps running cm FFN matmuls
        #      while the DVE grinds the h1 recurrence) ----
        cmv_t = vp.tile([P, 28], f32, name=f"cmv{li}", tag="cmv")
        nc.sync.dma_start(out=cmv_t, in_=cmv_d[li])
        wck_t = wp.tile([P, KD, F], b16, name=f"wck{li}", tag="wock")
        nc.sync.dma_start(out=wck_t, in_=wck_d[li].rearrange("k p d -> p k d"))
        wcv_t = wp.tile([P, KF, D], b16, name=f"wcv{li}", tag="wcv")
        nc.sync.dma_start(out=wcv_t, in_=wcv_d[li].rearrange("k p d -> p k d"))
        wcr_t = wp.tile([P, KD, D], b16, name=f"wcr{li}", tag="wrc")
        nc.sync.dma_start(out=wcr_t, in_=wcr_d[li].rearrange("k p d -> p k d"))

        sqc = alloc4("sqc", M, b16, tagp="sq")
        zc4 = [sb.tile([P, 2 + M], b16, name=f"zc{j}", tag=f"z{j}")
               for j in range(KD)]
        xkc = alloc4("xkc", M, b16, tagp="xkz")
        xrc = alloc4("xrc", M, b16, tagp="xrz")
        rf = alloc4("rf", M, b16, tagp="xvz")
        kf_t = [sb.tile([P, 4, 576], b16, name=f"kft{g}", tag=f"kf{g}")
                for g in range(4)]
        for j in range(KD):
            nc.vector.memset(zc4[j][:, 1:2], 0.0)

        def cm_mix(c0, cn):
            ln_apply_z(zc4, c0, cn)
            mixes(zc4, sqc, [(xkc, 0), (xrc, 4)], cmv_t, c0, cn, kas=True)

        def cm_half(c0, cn):
            cs = slice(c0, c0 + cn)
            z = zc4

            def rf_epi(m, ps):
                nc.scalar.activation(out=rf[m][:, c0:c0 + cn], in_=ps[:, :cn],
                                     func=AF.Tanh, scale=0.5,
                                     bias=cmv_t[:, 8 + m:9 + m])

            proj(xrc, wcr_t, c0, cn, rf_epi)
            for g in range(4):
                for jj in range(4):
                    fo = g * 4 + jj
                    kfp = pp.tile([P, 576], f32, name="kfp", bufs=3,
                                  tag="mm")
                    for (s0, sn) in _mm_slices(cn):
                        for kj in range(KD):
                            nc.tensor.matmul(
                                kfp[:, s0:s0 + sn],
                                lhsT=wck_t[:, kj, fo * P:(fo + 1) * P],
                                rhs=xkc[kj][:, c0 + s0:c0 + s0 + sn],
                                start=(kj == 0), stop=(kj == KD - 1))
                    nc.scalar.activation(out=kf_t[g][:, jj, :cn], in_=kfp[:, :cn],
                                         func=AF.Relu,
                                         bias=cmv_t[:, 12 + fo:13 + fo])
                nc.vector.tensor_tensor(kf_t[g][:, :, :cn], kf_t[g][:, :, :cn],
                                        kf_t[g][:, :, :cn], OP.mult)
            for m in range(KD):
                wvps = pp.tile([P, 576], f32, name="wvps", tag="mm", bufs=3)
                for (s0, sn) in _mm_slices(cn):
                    for kj in range(KF):
                        g, jj = kj // 4, kj % 4
                        nc.tensor.matmul(wvps[:, s0:s0 + sn],
                                         lhsT=wcv_t[:, kj, m * P:(m + 1) * P],
                                         rhs=kf_t[g][:, jj, s0:s0 + sn],
                                         start=(kj == 0), stop=(kj == KF - 1))
                t2 = vp.tile([P, 576], f32, name="t2", tag="t2s")
                # h += (tanh+1) * wvps  (0.5 folded into Wcv host-side)
                nc.vector.scalar_tensor_tensor(out=t2[:, :cn],
                                               in0=rf[m][:, cs], scalar=1.0,
                                               in1=wvps[:, :cn],
                                               op0=OP.add, op1=OP.mult)
                nc.gpsimd.dma_start(out=h[m][:, cs], in_=t2[:, :cn],
                                    accum_op=OP.add)

        tm_scans(*HALVES[0])
        tm_tail(*HALVES[0])
        tm_out(*HALVES[0])
        ln_stats(h, sqc, zc4, *HALVES[0])
        cm_mix(*HALVES[0])
        tm_scans(*HALVES[1])
        tm_tail(*HALVES[1])
        tm_out(*HALVES[1])
        cm_half(*HALVES[0])
        ln_stats(h, sqc, zc4, *HALVES[1])
        cm_mix(*HALVES[1])
        cm_half(*HALVES[1])

    # ---------------- final LN + head ----------------
    sq = alloc4("sq", M, b16)
    zf = [sb.tile([P, 2 + M], b16, name=f"z{j}", tag=f"z{j}") for j in range(KD)]
    whead_t = wp.tile([P, KD, OUT], b16, name="whead_t", tag="wock")
    nc.sync.dma_start(out=whead_t, in_=whead_d.rearrange("k p d -> p k d"))
    for (c0, cn) in HALVES:
        ln_stats(h, sq, zf, c0, cn)
        ln_apply_z(zf, c0, cn)
        for m in range(KD):
            ps = pp.tile([P, 576], f32, name="head_ps", tag="mm", bufs=3)
            for (s0, sn) in _mm_slices(cn):
                for kj in range(KD):
                    nc.tensor.matmul(
                        ps[:, s0:s0 + sn],
                        lhsT=whead_t[:, kj, m * P:(m + 1) * P],
                        rhs=zf[kj][:, 2 + c0 + s0:2 + c0 + s0 + sn],
                        start=(kj == 0), stop=(kj == KD - 1))
            nc.scalar.activation(out=h[m][:, c0:c0 + cn], in_=ps[:, :cn],
                                 func=AF.Identity, bias=headb_t[:, m:m + 1])
            nc.sync.dma_start(out=out_d[m][:, c0:c0 + cn],
                              in_=h[m][:, c0:c0 + cn])

    ctx.close()
    nc.compile()
    return nc


def _pack_cols(vec, kd=KD):
    """[kd*P] -> [P, kd] so that column j holds channels j*P..(j+1)*P-1."""
    return np.ascontiguousarray(vec.reshape(kd, P).T)


def _prep_inputs(inputs):
    bf16 = ml_dtypes.bfloat16
    f32 = np.float32
    inp = {k: np.asarray(v, dtype=f32) for k, v in inputs.items()}

    shared = {}
    wemb_p = np.zeros((P, D), f32)
    wemb_p[:E] = inp["emb_w"]
    shared["wemb"] = wemb_p.astype(bf16)
    shared["inv"] = np.concatenate(
        [_pack_cols(inp["emb_b"]), _pack_cols(inp["ln_in_w"]),
         _pack_cols(inp["ln_in_b"])], axis=1).astype(f32)

    def fold(w_vec, mat):
        return (w_vec[:, None] * mat)

    wk = np.stack([fold(inp["ln0_w"][i], inp["tm_wk"][i]) for i in range(L)])
    # 0.5 folded into Wv: sigmoid(r) = 0.5*(tanh(r/2)+1), the 0.5 rides on v
    wv = np.stack([0.5 * fold(inp["ln0_w"][i], inp["tm_wv"][i])
                   for i in range(L)])
    wr = np.stack([fold(inp["ln0_w"][i], inp["tm_wr"][i]) for i in range(L)])
    wo = inp["tm_wo"]
    wck = np.stack([fold(inp["ln1_w"][i], inp["cm_wk"][i]) for i in range(L)])
    wcr = np.stack([fold(inp["ln1_w"][i], inp["cm_wr"][i]) for i in range(L)])
    wcv = 0.5 * inp["cm_wv"]

    shared["wk"] = wk.reshape(L, KD, P, D).astype(bf16)
    shared["wv"] = wv.reshape(L, KD, P, D).astype(bf16)
    shared["wr"] = wr.reshape(L, KD, P, D).astype(bf16)
    shared["wo"] = wo.reshape(L, KD, P, D).astype(bf16)
    shared["wck"] = wck.reshape(L, KD, P, F).astype(bf16)
    shared["wcv"] = wcv.reshape(L, KF, P, D).astype(bf16)
    shared["wcr"] = wcr.reshape(L, KD, P, D).astype(bf16)
    shared["whead"] = (inp["ln_out_w"][:, None] * inp["head_w"]).reshape(
        KD, P, OUT).astype(bf16)
    shared["headb"] = _pack_cols(inp["ln_out_b"] @ inp["head_w"]).astype(f32)

    tmv = np.zeros((L, P, 32), f32)
    cmv = np.zeros((L, P, 28), f32)
    for i in range(L):
        ew = np.exp(-np.exp(inp["tm_decay"][i]))
        tmv[i, :, 0:4] = _pack_cols(inp["tm_mix_k"][i] - 1.0)
        tmv[i, :, 4:8] = _pack_cols(inp["tm_mix_v"][i] - 1.0)
        tmv[i, :, 8:12] = _pack_cols(inp["tm_mix_r"][i] - 1.0)
        tmv[i, :, 12:16] = _pack_cols(ew)
        tmv[i, :, 16:20] = _pack_cols(np.exp(inp["tm_first"][i]))
        tmv[i, :, 20:24] = _pack_cols(inp["ln0_b"][i] @ inp["tm_wk"][i])
        tmv[i, :, 24:28] = _pack_cols(0.5 * (inp["ln0_b"][i] @ inp["tm_wv"][i]))
        tmv[i, :, 28:32] = _pack_cols(0.5 * (inp["ln0_b"][i] @ inp["tm_wr"][i]))
        cmv[i, :, 0:4] = _pack_cols(inp["cm_mix_k"][i] - 1.0)
        cmv[i, :, 4:8] = _pack_cols(inp["cm_mix_r"][i] - 1.0)
        cmv[i, :, 8:12] = _pack_cols(0.5 * (inp["ln1_b"][i] @ inp["cm_wr"][i]))
        cmv[i, :, 12:28] = _pack_cols(inp["ln1_b"][i] @ inp["cm_wk"][i], kd=KF)
    shared["tmv"] = tmv
    shared["cmv"] = cmv

    in_maps = []
    x = inp["x"]
    for c in range(N_CORES):
        b, half = c // 2, c % 2
        t0 = 0 if half == 0 else T - M
        x_sl = np.zeros((P, M), f32)
        x_sl[:E] = x[b, t0:t0 + M].T
        m = dict(shared)
        m["xT"] = x_sl.astype(bf16)
        in_maps.append(m)
    return in_maps


TRACE = False  # set by test harness to capture an NTFF profile


def kernel(**inputs):
    from concourse import bass_utils

    if "nc" not in _CACHE:
        _CACHE["nc"] = _build_bass()
    nc = _CACHE["nc"]
    in_maps = _prep_inputs(inputs)
    res = bass_utils.run_bass_kernel_spmd(nc, in_maps, core_ids=list(range(N_CORES)),
                                          trace=TRACE)
    _CACHE["last_res"] = res
    out = np.zeros((B, T, OUT), np.float32)
    for c in range(N_CORES):
        b, half = c // 2, c % 2
        oT = res.results[c]["outT"].reshape(D, M)  # [channels, tokens]
        o = np.ascontiguousarray(oT.T)             # [tokens, channels]
        if half == 0:
            out[b, :S_SPLIT] = o[:S_SPLIT]
        else:
            out[b, S_SPLIT:] = o[M - (T - S_SPLIT):]
    return out


# revision 25
# speedup vs baseline: 1.2010x; 1.2010x over previous
"""RWKV-style CausalEventModel kernel for 8 Trainium2 NeuronCores.

Strategy (zero cross-core communication):
  - Data-parallel over batch (B=4) x 2-way sequence split per batch = 8 cores.
  - Each core runs the FULL model on M=1088 tokens in channel-major layout
    ([D partitions, tokens free]).  The second-half core starts W=128 tokens
    early with zero initial WKV state; the per-channel decay makes the
    missing-prefix contribution negligible by the output region.
  - Two token half-blocks (512 / 576) are software-pipelined through every
    layer phase; the WKV recurrence state chains across halves via the scan's
    `initial` operand.

V2 performance rework (vs. the first working version):
  - One ACT table-set discipline: only the LN-row Sqrt swaps table sets
    (2 swaps per phase; stats for both halves batched at phase start);
    Exp/Tanh/Square/Relu/Copy/Identity all live in exp_and_others.
  - sigmoid(x) = 0.5*(tanh(x/2)+1): computed as Tanh on the ACT engine with
    the 0.5 folded into Wv / Wcv host-side, so no sigmoid table set is needed.
  - WKV assembly fused into scalar_tensor_tensor with e^u as the per-channel
    scalar: num = ekv*e^u + A_shift, den = ek*e^u + B_shift.  1/den uses the
    single-instruction DVE reciprocal_approx_fast (fp32, ~18 bits).
  - Channel-mix relu()^2: ACT Relu epilogue + one in-place 4-wide DVE
    tensor_tensor square per group at bf16 2x rate.
  - Elementwise ops keep bf16 step-1 4B-aligned operands wherever possible so
    DVE runs in its 2x packed mode; z/d bulk ops ride on GpSimd to keep DVE
    free for scans/STT.
"""
import numpy as np
import ml_dtypes

B, T, E, D, F, L, OUT = 4, 2048, 4, 512, 2048, 8, 512
P = 128
KD = D // P          # 4
KF = F // P          # 16
W_WARM = 128
M = (T + W_WARM) // 2        # 1088 tokens per core
S_SPLIT = M                  # first-half output rows
HALVES = [(0, 512), (512, M - 512)]          # token half-blocks per core
N_CORES = 8
EPS = 1e-5

_CACHE = {}


def _mm_slices(cn):
    """Output-column slices (relative to a PSUM tile start) that keep each
    matmul's output inside one 2KB PSUM bank."""
    out = [(0, min(512, cn))]
    if cn > 512:
        out.append((512, cn - 512))
    return out


def _build_bass():
    import concourse.bass as bass
    import concourse.bacc as bacc
    import concourse.mybir as mybir
    import concourse.tile as tile
    from contextlib import ExitStack

    f32 = mybir.dt.float32
    f32r = mybir.dt.float32r
    b16 = mybir.dt.bfloat16
    AF = mybir.ActivationFunctionType
    OP = mybir.AluOpType

    nc = bacc.Bacc("TRN2", target_bir_lowering=False, debug=False)

    # ---------------- DRAM tensors ----------------
    xT_d = nc.dram_tensor("xT", [P, M], b16, kind="ExternalInput")
    wemb_d = nc.dram_tensor("wemb", [P, D], b16, kind="ExternalInput")
    inv_d = nc.dram_tensor("inv", [P, 12], f32, kind="ExternalInput")
    wk_d = nc.dram_tensor("wk", [L, KD, P, D], b16, kind="ExternalInput")
    wv_d = nc.dram_tensor("wv", [L, KD, P, D], b16, kind="ExternalInput")
    wr_d = nc.dram_tensor("wr", [L, KD, P, D], b16, kind="ExternalInput")
    wo_d = nc.dram_tensor("wo", [L, KD, P, D], b16, kind="ExternalInput")
    wck_d = nc.dram_tensor("wck", [L, KD, P, F], b16, kind="ExternalInput")
    wcv_d = nc.dram_tensor("wcv", [L, KF, P, D], b16, kind="ExternalInput")
    wcr_d = nc.dram_tensor("wcr", [L, KD, P, D], b16, kind="ExternalInput")
    whead_d = nc.dram_tensor("whead", [KD, P, OUT], b16, kind="ExternalInput")
    tmv_d = nc.dram_tensor("tmv", [L, P, 32], f32, kind="ExternalInput")
    cmv_d = nc.dram_tensor("cmv", [L, P, 28], f32, kind="ExternalInput")
    headb_d = nc.dram_tensor("headb", [P, KD], f32, kind="ExternalInput")
    out_d = nc.dram_tensor("outT", [KD, P, M], f32, kind="ExternalOutput")

    ctx = ExitStack()
    tc = ctx.enter_context(tile.TileContext(nc))
    sb = ctx.enter_context(tc.tile_pool(name="sb", bufs=1))
    vp = ctx.enter_context(tc.tile_pool(name="vp", bufs=2))
    wp = ctx.enter_context(tc.tile_pool(name="wp", bufs=1))
    pp = ctx.enter_context(tc.tile_pool(name="pp", bufs=2, space="PSUM"))

    # persistent tiles
    h = [sb.tile([P, M], f32, name=f"h{j}", tag=f"h{j}") for j in range(KD)]
    ones_b = sb.tile([P, 1], b16, name="ones_b", tag="ones_b")
    nc.vector.memset(ones_b, 1.0)
    ones_f = sb.tile([P, 1], f32, name="ones_f", tag="ones_f")
    nc.vector.memset(ones_f, 1.0)
    ones_row = sb.tile([1, P], b16, name="ones_row", tag="ones_row")
    nc.vector.memset(ones_row, 1.0)
    srowA = sb.tile([1, M], f32, name="srowA", tag="srowA")
    srowB = sb.tile([1, M], f32, name="srowB", tag="srowB")
    rb0 = sb.tile([1, M], b16, name="rb0", tag="rb0")
    rb1 = sb.tile([1, M], b16, name="rb1", tag="rb1")
    eps_col = sb.tile([P, 1], f32, name="eps_col", tag="eps_col")
    nc.vector.memset(eps_col, EPS)
    rstd_sb = sb.tile([P, M], b16, name="rstd_sb", tag="rstd_sb")
    mean_sb = sb.tile([P, M], b16, name="mean_sb", tag="mean_sb")

    def alloc4(prefix, width, dtype, pool=sb, tagp=None):
        tagp = tagp or prefix
        return [pool.tile([P, width], dtype, name=f"{prefix}{j}", tag=f"{tagp}{j}")
                for j in range(KD)]

    def ln_stats(h_tiles, sq_tiles, z_tiles, c0, cn):
        """Per-token mean/rstd of h[:, c0:c0+cn] over 512 channels into
        rstd_sb / mean_sb (bf16 broadcast tiles, absolute token columns).
        Leaves a bf16 copy of h in z[:, 2+c0:] for the sum matmul; LN-apply
        then normalizes z in place.  z tiles are [P, 2+M] so the partition
        pitch stays 4B-aligned (DVE 2x packed mode eligibility)."""
        cs = slice(c0, c0 + cn)
        zs = slice(2 + c0, 2 + c0 + cn)
        for j in range(KD):
            nc.vector.tensor_copy(out=z_tiles[j][:, zs], in_=h_tiles[j][:, cs])
        # sq = h^2 in bf16 (Square is in every ACT table set)
        for j in range(KD):
            nc.scalar.activation(out=sq_tiles[j][:, cs], in_=h_tiles[j][:, cs],
                                 func=AF.Square)
        for (s0, sn) in _mm_slices(cn):
            a0 = c0 + s0
            sl = slice(a0, a0 + sn)
            sum_ps = pp.tile([1, 512], f32, name="sum_ps", tag="st")
            sq_ps = pp.tile([1, 512], f32, name="sq_ps", tag="st")
            for j in range(KD):
                nc.tensor.matmul(sum_ps[0:1, :sn],
                                 lhsT=ones_b,
                                 rhs=z_tiles[j][:, 2 + a0:2 + a0 + sn],
                                 start=(j == 0), stop=(j == KD - 1))
            for j in range(KD):
                nc.tensor.matmul(sq_ps[0:1, :sn], lhsT=ones_b,
                                 rhs=sq_tiles[j][:, sl],
                                 start=(j == 0), stop=(j == KD - 1))
            # row math on partition 0: var = sqsum/D - (sum/D)^2
            sB = srowB[0:1, sl]
            nc.scalar.activation(out=sB, in_=sum_ps[0:1, :sn], func=AF.Square,
                                 scale=1.0 / D)
            nc.vector.scalar_tensor_tensor(out=srowA[0:1, sl],
                                           in0=sq_ps[0:1, :sn],
                                           scalar=1.0 / D, in1=sB,
                                           op0=OP.mult, op1=OP.subtract)
            nc.scalar.activation(out=srowA[0:1, sl], in_=srowA[0:1, sl],
                                 func=AF.Sqrt, bias=eps_col[0:1, :])
            nc.vector.reciprocal_approx_fast(out=srowB[0:1, sl],
                                             in_=srowA[0:1, sl])
            with nc.allow_low_precision(reason="per-token rstd in bf16"):
                nc.vector.tensor_copy(out=rb0[0:1, sl], in_=srowB[0:1, sl])
                nc.scalar.activation(out=rb1[0:1, sl], in_=sum_ps[0:1, :sn],
                                     func=AF.Copy, scale=1.0 / D)
            # broadcast across partitions: K=1 matmul -> PSUM -> bf16 SBUF
            bc_ps = pp.tile([P, 576], f32, name="bc_ps", tag="mm", bufs=3)
            bc_ps2 = pp.tile([P, 576], f32, name="bc_ps2", tag="mm", bufs=3)
            nc.tensor.matmul(bc_ps[:, :sn], lhsT=ones_row, rhs=rb0[0:1, sl],
                             start=True, stop=True)
            nc.tensor.matmul(bc_ps2[:, :sn], lhsT=ones_row, rhs=rb1[0:1, sl],
                             start=True, stop=True)
            nc.scalar.activation(out=rstd_sb[:, sl], in_=bc_ps[:, :sn],
                                 func=AF.Copy)
            nc.scalar.activation(out=mean_sb[:, sl], in_=bc_ps2[:, :sn],
                                 func=AF.Copy)

    def ka(src_tile, c0):
        """HAM keep-alive: a tiny matmul that depends on a just-produced
        elementwise result, so the PE sees activity inside long vector-only
        windows and its clock stays at K=8/8 (2.4 GHz)."""
        ka_ps = pp.tile([1, 512], f32, name="ka_ps", tag="st")
        nc.tensor.matmul(ka_ps[0:1, :64], lhsT=ones_b,
                         rhs=src_tile[:, c0:c0 + 64],
                         start=True, stop=True)

    def ln_apply_z(z_tiles, c0, cn):
        """z = (z - mean)*rstd in place on the half-block (GpSimd)."""
        cs = slice(c0, c0 + cn)
        zs = slice(2 + c0, 2 + c0 + cn)
        for j in range(KD):
            eng = nc.vector if j % 2 == 0 else nc.gpsimd
            eng.tensor_tensor(z_tiles[j][:, zs], z_tiles[j][:, zs],
                              mean_sb[:, cs], OP.subtract)
        for j in range(KD):
            eng = nc.vector if j % 2 == 0 else nc.gpsimd
            eng.tensor_tensor(z_tiles[j][:, zs], z_tiles[j][:, zs],
                              rstd_sb[:, cs], OP.mult)
            if j % 2 == 1:
                ka(z_tiles[j], 2 + c0)

    def mixes(z_t, d_t, outs_scalars, vec_t, c0, cn, kas=False):
        """out = mix*z + (1-mix)*z_sh = z + (mix-1)*d with d = z - z_sh.
        The host packs (mix-1) so both STT tensor operands read at aligned
        even offsets -> DVE 2x.  d lives in the dead sq tiles."""
        for j in range(KD):
            zc = z_t[j][:, 2 + c0:2 + c0 + cn]
            zsh = z_t[j][:, 1 + c0:1 + c0 + cn]
            eng = nc.vector if j % 2 == 0 else nc.gpsimd
            eng.tensor_tensor(d_t[j][:, c0:c0 + cn], zc, zsh, OP.subtract)
            if kas and j % 2 == 1:
                ka(d_t[j], c0)
        for (out_tiles, col) in outs_scalars:
            for j in range(KD):
                zc = z_t[j][:, 2 + c0:2 + c0 + cn]
                nc.vector.scalar_tensor_tensor(out=out_tiles[j][:, c0:c0 + cn],
                                               in0=d_t[j][:, c0:c0 + cn],
                                               scalar=vec_t[:, col + j:col + j + 1],
                                               in1=zc, op0=OP.mult, op1=OP.add)

    def proj(rhs_tiles, w_t, c0, cn, epilogue):
        """epilogue(m, ps) consumes the [P, cn] PSUM of output tile m."""
        for m in range(KD):
            ps = pp.tile([P, 576], f32, name="proj_ps", tag="mm", bufs=3)
            for (s0, sn) in _mm_slices(cn):
                for kj in range(KD):
                    nc.tensor.matmul(
                        ps[:, s0:s0 + sn],
                        lhsT=w_t[:, kj, m * P:(m + 1) * P],
                        rhs=rhs_tiles[kj][:, c0 + s0:c0 + s0 + sn],
                        start=(kj == 0), stop=(kj == KD - 1))
            epilogue(m, ps)

    # ---------------- embedding ----------------
    xt = sb.tile([P, M], b16, name="xt", tag="xt")
    nc.gpsimd.dma_start(out=xt, in_=xT_d[:, :])
    wemb_t = sb.tile([P, D], b16, name="wemb_t", tag="wemb_t")
    nc.gpsimd.dma_start(out=wemb_t, in_=wemb_d[:, :])
    inv_t = sb.tile([P, 12], f32, name="inv_t", tag="inv_t")
    nc.gpsimd.dma_start(out=inv_t, in_=inv_d[:, :])
    headb_t = sb.tile([P, KD], f32, name="headb_t", tag="headb_t")
    nc.gpsimd.dma_start(out=headb_t, in_=headb_d[:, :])

    for (c0, cn) in HALVES:
        for m in range(KD):
            ps = pp.tile([P, 576], f32, name=f"emb_ps{m}", tag="mm", bufs=3)
            for (s0, sn) in _mm_slices(cn):
                nc.tensor.matmul(ps[:, s0:s0 + sn],
                                 lhsT=wemb_t[:, m * P:(m + 1) * P],
                                 rhs=xt[:, c0 + s0:c0 + s0 + sn],
                                 start=True, stop=True)
            nc.scalar.activation(out=h[m][:, c0:c0 + cn], in_=ps[:, :cn],
                                 func=AF.Identity, bias=inv_t[:, m:m + 1])

    # ln_in (explicit w/b application since h is the residual stream)
    sq = alloc4("sq", M, b16)
    z = [sb.tile([P, 2 + M], b16, name=f"z{j}", tag=f"z{j}") for j in range(KD)]
    for j in range(KD):
        nc.vector.memset(z[j][:, 1:2], 0.0)
    for (c0, cn) in HALVES:
        ln_stats(h, sq, z, c0, cn)
        ln_apply_z(z, c0, cn)
        for j in range(KD):
            nc.vector.tensor_scalar(out=h[j][:, c0:c0 + cn],
                                    in0=z[j][:, 2 + c0:2 + c0 + cn],
                                    scalar1=inv_t[:, 4 + j:5 + j],
                                    scalar2=inv_t[:, 8 + j:9 + j],
                                    op0=OP.mult, op1=OP.add)

    # ---------------- layers ----------------
    for li in range(L):
        tmv_t = vp.tile([P, 32], f32, name=f"tmv{li}", tag="tmv")
        nc.sync.dma_start(out=tmv_t, in_=tmv_d[li])
        wk_t = wp.tile([P, KD, D], b16, name=f"wk{li}", tag="wk", bufs=2)
        nc.sync.dma_start(out=wk_t, in_=wk_d[li].rearrange("k p d -> p k d"))
        wv_t = wp.tile([P, KD, D], b16, name=f"wv{li}", tag="wv")
        nc.sync.dma_start(out=wv_t, in_=wv_d[li].rearrange("k p d -> p k d"))
        wr_t = wp.tile([P, KD, D], b16, name=f"wr{li}", tag="wrc")
        nc.sync.dma_start(out=wr_t, in_=wr_d[li].rearrange("k p d -> p k d"))
        wo_t = wp.tile([P, KD, D], b16, name=f"wo{li}", tag="wock")
        nc.sync.dma_start(out=wo_t, in_=wo_d[li].rearrange("k p d -> p k d"))

        # ---- time mixing ----
        sq = alloc4("sq", M, b16)
        z = [sb.tile([P, 2 + M], b16, name=f"z{j}", tag=f"z{j}")
             for j in range(KD)]
        xkz = alloc4("xkz", M, b16)
        xvz = alloc4("xvz", M, b16)
        xrz = alloc4("xrz", M, b16)
        # ek/vv/rr/sc live in the channel-mix kf slots (disjoint lifetime)
        ek = alloc4("ek", M, b16)
        vv = alloc4("vv", M, b16)
        rr = alloc4("rr", M, b16)
        sc = [sb.tile([P, 2 + M], b16, name=f"sc{j}", tag=f"sc{j}")
              for j in range(KD)]
        scB = [sb.tile([P, 2 + M], b16, name=f"scB{j}", tag=f"t2_{j}")
               for j in range(KD)]
        den = [sb.tile([P, 576], f32, name=f"den{j}", tag=f"den{j}")
               for j in range(KD)]
        for j in range(KD):
            nc.vector.memset(z[j][:, 1:2], 0.0)
            nc.vector.memset(sc[j][:, 0:1], 0.0)
            nc.vector.memset(scB[j][:, 0:1], 0.0)

        # stats for both halves first: one Rsqrt table-set period per phase
        for (c0, cn) in HALVES:
            ln_stats(h, sq, z, c0, cn)

        def tm_mix_proj(c0, cn):
            ln_apply_z(z, c0, cn)
            mixes(z, sq, [(xkz, 0), (xvz, 4), (xrz, 8)], tmv_t, c0, cn,
                  kas=True)

            def k_epi(m, ps):
                nc.scalar.activation(out=ek[m][:, c0:c0 + cn], in_=ps[:, :cn],
                                     func=AF.Exp,
                                     bias=tmv_t[:, 20 + m:21 + m])

            def v_epi(m, ps):
                nc.scalar.activation(out=vv[m][:, c0:c0 + cn], in_=ps[:, :cn],
                                     func=AF.Identity,
                                     bias=tmv_t[:, 24 + m:25 + m])

            def r_epi(m, ps):
                nc.scalar.activation(out=rr[m][:, c0:c0 + cn], in_=ps[:, :cn],
                                     func=AF.Tanh, scale=0.5,
                                     bias=tmv_t[:, 28 + m:29 + m])

            proj(xkz, wk_t, c0, cn, k_epi)
            proj(xvz, wv_t, c0, cn, v_epi)
            proj(xrz, wr_t, c0, cn, r_epi)

        def tm_scans(c0, cn):
            cs = slice(c0, c0 + cn)
            for j in range(KD):
                # ekv into the xvz slot (v-proj has fully consumed it)
                nc.vector.tensor_tensor(xvz[j][:, cs], ek[j][:, cs],
                                        vv[j][:, cs], OP.mult)
            for j in range(KD):
                ew_ap = tmv_t[:, 12 + j:13 + j].to_broadcast([P, cn])
                init = 0.0 if c0 == 0 else sc[j][:, c0:c0 + 1]
                nc.vector.tensor_tensor_scan(out=sc[j][:, 1 + c0:1 + c0 + cn],
                                             data0=ew_ap, data1=xvz[j][:, cs],
                                             initial=init,
                                             op0=OP.mult, op1=OP.add)
                ka(sc[j], c0)
            for j in range(KD):
                ew_ap = tmv_t[:, 12 + j:13 + j].to_broadcast([P, cn])
                init = 0.0 if c0 == 0 else scB[j][:, c0:c0 + 1]
                nc.vector.tensor_tensor_scan(out=scB[j][:, 1 + c0:1 + c0 + cn],
                                             data0=ew_ap, data1=ek[j][:, cs],
                                             initial=init,
                                             op0=OP.mult, op1=OP.add)
                if j % 2 == 1:
                    ka(scB[j], c0)

        def tm_tail(c0, cn):
            cs = slice(c0, c0 + cn)
            for j in range(KD):
                # num = ekv*e^u + A_shift  (overwrite vv; all operands at
                # aligned even offsets -> DVE 2x)
                nc.vector.scalar_tensor_tensor(out=vv[j][:, cs],
                                               in0=xvz[j][:, cs],
                                               scalar=tmv_t[:, 16 + j:17 + j],
                                               in1=sc[j][:, c0:c0 + cn],
                                               op0=OP.mult, op1=OP.add)
                if j % 2 == 1:
                    ka(vv[j], c0)
            for j in range(KD):
                # den = ek*e^u + B_shift (fp32 for the fast reciprocal)
                nc.vector.scalar_tensor_tensor(out=den[j][:, :cn],
                                               in0=ek[j][:, cs],
                                               scalar=tmv_t[:, 16 + j:17 + j],
                                               in1=scB[j][:, c0:c0 + cn],
                                               op0=OP.mult, op1=OP.add)
            for j in range(KD):
                nc.vector.reciprocal_approx_fast(out=den[j][:, :cn],
                                                 in_=den[j][:, :cn])
            for j in range(KD):
                # rn = (tanh+1)*recip  (0.5 folded into Wv host-side)
                nc.vector.scalar_tensor_tensor(out=rr[j][:, cs],
                                               in0=rr[j][:, cs], scalar=1.0,
                                               in1=den[j][:, :cn],
                                               op0=OP.add, op1=OP.mult)
            for j in range(KD):
                # rwkv = num * (rr_scaled * recip) -> k-proj's dead xkz slot
                nc.vector.tensor_tensor(xkz[j][:, cs], vv[j][:, cs],
                                        rr[j][:, cs], OP.mult)

        def tm_out(c0, cn):
            cs = slice(c0, c0 + cn)

            def o_epi(m, ps):
                nc.vector.tensor_tensor(h[m][:, cs], h[m][:, cs], ps[:, :cn],
                                        OP.add)

            proj(xkz, wo_t, c0, cn, o_epi)

        tm_mix_proj(*HALVES[0])
        tm_mix_proj(*HALVES[1])

        # ---- channel mixing, interleaved with the tm h1 recurrence ----
        # Reuses the tm tile objects: xkc->xkz slots, xrc->xrz, rf->xvz,
        # and the same z/sq tiles, so only slice-level deps are created.
        cmv_t = vp.tile([P, 28], f32, name=f"cmv{li}", tag="cmv")
        nc.sync.dma_start(out=cmv_t, in_=cmv_d[li])
        wck_t = wp.tile([P, KD, F], b16, name=f"wck{li}", tag="wck")
        nc.sync.dma_start(out=wck_t, in_=wck_d[li].rearrange("k p d -> p k d"))
        wcv_t = wp.tile([P, KF, D], b16, name=f"wcv{li}", tag="wcv")
        nc.sync.dma_start(out=wcv_t, in_=wcv_d[li].rearrange("k p d -> p k d"))
        wcr_t = wp.tile([P, KD, D], b16, name=f"wcr{li}", tag="wrc")
        nc.sync.dma_start(out=wcr_t, in_=wcr_d[li].rearrange("k p d -> p k d"))

        xkc, xrc, rf = xkz, xrz, xvz
        kf_t = [sb.tile([P, 4, 576], b16, name=f"kft{g}", tag=f"kf{g}")
                for g in range(4)]

        def cm_mix(c0, cn):
            ln_apply_z(z, c0, cn)
            mixes(z, sq, [(xkc, 0), (xrc, 4)], cmv_t, c0, cn, kas=True)

        def cm_rfck(c0, cn):
            def rf_epi(m, ps):
                nc.scalar.activation(out=rf[m][:, c0:c0 + cn], in_=ps[:, :cn],
                                     func=AF.Tanh, scale=0.5,
                                     bias=cmv_t[:, 8 + m:9 + m])

            proj(xrc, wcr_t, c0, cn, rf_epi)
            for g in range(4):
                for jj in range(4):
                    fo = g * 4 + jj
                    kfp = pp.tile([P, 576], f32, name="kfp", bufs=3,
                                  tag="mm")
                    for (s0, sn) in _mm_slices(cn):
                        for kj in range(KD):
                            nc.tensor.matmul(
                                kfp[:, s0:s0 + sn],
                                lhsT=wck_t[:, kj, fo * P:(fo + 1) * P],
                                rhs=xkc[kj][:, c0 + s0:c0 + s0 + sn],
                                start=(kj == 0), stop=(kj == KD - 1))
                    nc.scalar.activation(out=kf_t[g][:, jj, :cn], in_=kfp[:, :cn],
                                         func=AF.Relu,
                                         bias=cmv_t[:, 12 + fo:13 + fo])
                nc.vector.tensor_tensor(kf_t[g][:, :, :cn], kf_t[g][:, :, :cn],
                                        kf_t[g][:, :, :cn], OP.mult)

        def cm_wv(c0, cn):
            cs = slice(c0, c0 + cn)
            for m in range(KD):
                wvps = pp.tile([P, 576], f32, name="wvps", tag="mm", bufs=3)
                for (s0, sn) in _mm_slices(cn):
                    for kj in range(KF):
                        g, jj = kj // 4, kj % 4
                        nc.tensor.matmul(wvps[:, s0:s0 + sn],
                                         lhsT=wcv_t[:, kj, m * P:(m + 1) * P],
                                         rhs=kf_t[g][:, jj, s0:s0 + sn],
                                         start=(kj == 0), stop=(kj == KF - 1))
                t2 = vp.tile([P, 576], f32, name="t2", tag="t2s")
                # h += (tanh+1) * wvps  (0.5 folded into Wcv host-side)
                nc.vector.scalar_tensor_tensor(out=t2[:, :cn],
                                               in0=rf[m][:, cs], scalar=1.0,
                                               in1=wvps[:, :cn],
                                               op0=OP.add, op1=OP.mult)
                nc.gpsimd.dma_start(out=h[m][:, cs], in_=t2[:, :cn],
                                    accum_op=OP.add)

        tm_scans(*HALVES[0])
        tm_tail(*HALVES[0])
        tm_out(*HALVES[0])
        ln_stats(h, sq, z, *HALVES[0])
        cm_mix(*HALVES[0])
        tm_scans(*HALVES[1])
        cm_rfck(*HALVES[0])
        tm_tail(*HALVES[1])
        tm_out(*HALVES[1])
        cm_wv(*HALVES[0])
        ln_stats(h, sq, z, *HALVES[1])
        cm_mix(*HALVES[1])
        cm_rfck(*HALVES[1])
        cm_wv(*HALVES[1])

    # ---------------- final LN + head ----------------
    sq = alloc4("sq", M, b16)
    zf = [sb.tile([P, 2 + M], b16, name=f"z{j}", tag=f"z{j}") for j in range(KD)]
    whead_t = wp.tile([P, KD, OUT], b16, name="whead_t", tag="wock")
    nc.sync.dma_start(out=whead_t, in_=whead_d.rearrange("k p d -> p k d"))
    for (c0, cn) in HALVES:
        ln_stats(h, sq, zf, c0, cn)
        ln_apply_z(zf, c0, cn)
        for m in range(KD):
            ps = pp.tile([P, 576], f32, name="head_ps", tag="mm", bufs=3)
            for (s0, sn) in _mm_slices(cn):
                for kj in range(KD):
                    nc.tensor.matmul(
                        ps[:, s0:s0 + sn],
                        lhsT=whead_t[:, kj, m * P:(m + 1) * P],
                        rhs=zf[kj][:, 2 + c0 + s0:2 + c0 + s0 + sn],
                        start=(kj == 0), stop=(kj == KD - 1))
            nc.scalar.activation(out=h[m][:, c0:c0 + cn], in_=ps[:, :cn],
                                 func=AF.Identity, bias=headb_t[:, m:m + 1])
            nc.sync.dma_start(out=out_d[m][:, c0:c0 + cn],
                              in_=h[m][:, c0:c0 + cn])

    ctx.close()
    nc.compile()
    return nc


def _pack_cols(vec, kd=KD):
    """[kd*P] -> [P, kd] so that column j holds channels j*P..(j+1)*P-1."""
    return np.ascontiguousarray(vec.reshape(kd, P).T)


def _prep_inputs(inputs):
    bf16 = ml_dtypes.bfloat16
    f32 = np.float32
    inp = {k: np.asarray(v, dtype=f32) for k, v in inputs.items()}

    shared = {}
    wemb_p = np.zeros((P, D), f32)
    wemb_p[:E] = inp["emb_w"]
    shared["wemb"] = wemb_p.astype(bf16)
    shared["inv"] = np.concatenate(
        [_pack_cols(inp["emb_b"]), _pack_cols(inp["ln_in_w"]),
         _pack_cols(inp["ln_in_b"])], axis=1).astype(f32)

    def fold(w_vec, mat):
        return (w_vec[:, None] * mat)

    wk = np.stack([fold(inp["ln0_w"][i], inp["tm_wk"][i]) for i in range(L)])
    # 0.5 folded into Wv: sigmoid(r) = 0.5*(tanh(r/2)+1), the 0.5 rides on v
    wv = np.stack([0.5 * fold(inp["ln0_w"][i], inp["tm_wv"][i])
                   for i in range(L)])
    wr = np.stack([fold(inp["ln0_w"][i], inp["tm_wr"][i]) for i in range(L)])
    wo = inp["tm_wo"]
    wck = np.stack([fold(inp["ln1_w"][i], inp["cm_wk"][i]) for i in range(L)])
    wcr = np.stack([fold(inp["ln1_w"][i], inp["cm_wr"][i]) for i in range(L)])
    wcv = 0.5 * inp["cm_wv"]

    shared["wk"] = wk.reshape(L, KD, P, D).astype(bf16)
    shared["wv"] = wv.reshape(L, KD, P, D).astype(bf16)
    shared["wr"] = wr.reshape(L, KD, P, D).astype(bf16)
    shared["wo"] = wo.reshape(L, KD, P, D).astype(bf16)
    shared["wck"] = wck.reshape(L, KD, P, F).astype(bf16)
    shared["wcv"] = wcv.reshape(L, KF, P, D).astype(bf16)
    shared["wcr"] = wcr.reshape(L, KD, P, D).astype(bf16)
    shared["whead"] = (inp["ln_out_w"][:, None] * inp["head_w"]).reshape(
        KD, P, OUT).astype(bf16)
    shared["headb"] = _pack_cols(inp["ln_out_b"] @ inp["head_w"]).astype(f32)

    tmv = np.zeros((L, P, 32), f32)
    cmv = np.zeros((L, P, 28), f32)
    for i in range(L):
        ew = np.exp(-np.exp(inp["tm_decay"][i]))
        tmv[i, :, 0:4] = _pack_cols(inp["tm_mix_k"][i] - 1.0)
        tmv[i, :, 4:8] = _pack_cols(inp["tm_mix_v"][i] - 1.0)
        tmv[i, :, 8:12] = _pack_cols(inp["tm_mix_r"][i] - 1.0)
        tmv[i, :, 12:16] = _pack_cols(ew)
        tmv[i, :, 16:20] = _pack_cols(np.exp(inp["tm_first"][i]))
        tmv[i, :, 20:24] = _pack_cols(inp["ln0_b"][i] @ inp["tm_wk"][i])
        tmv[i, :, 24:28] = _pack_cols(0.5 * (inp["ln0_b"][i] @ inp["tm_wv"][i]))
        tmv[i, :, 28:32] = _pack_cols(0.5 * (inp["ln0_b"][i] @ inp["tm_wr"][i]))
        cmv[i, :, 0:4] = _pack_cols(inp["cm_mix_k"][i] - 1.0)
        cmv[i, :, 4:8] = _pack_cols(inp["cm_mix_r"][i] - 1.0)
        cmv[i, :, 8:12] = _pack_cols(0.5 * (inp["ln1_b"][i] @ inp["cm_wr"][i]))
        cmv[i, :, 12:28] = _pack_cols(inp["ln1_b"][i] @ inp["cm_wk"][i], kd=KF)
    shared["tmv"] = tmv
    shared["cmv"] = cmv

    in_maps = []
    x = inp["x"]
    for c in range(N_CORES):
        b, half = c // 2, c % 2
        t0 = 0 if half == 0 else T - M
        x_sl = np.zeros((P, M), f32)
        x_sl[:E] = x[b, t0:t0 + M].T
        m = dict(shared)
        m["xT"] = x_sl.astype(bf16)
        in_maps.append(m)
    return in_maps


TRACE = False  # set by test harness to capture an NTFF profile


def kernel(**inputs):
    from concourse import bass_utils

    if "nc" not in _CACHE:
        _CACHE["nc"] = _build_bass()
    nc = _CACHE["nc"]
    in_maps = _prep_inputs(inputs)
    res = bass_utils.run_bass_kernel_spmd(nc, in_maps, core_ids=list(range(N_CORES)),
                                          trace=TRACE)
    _CACHE["last_res"] = res
    out = np.zeros((B, T, OUT), np.float32)
    for c in range(N_CORES):
        b, half = c // 2, c % 2
        oT = res.results[c]["outT"].reshape(D, M)  # [channels, tokens]
        o = np.ascontiguousarray(oT.T)             # [tokens, channels]
        if half == 0:
            out[b, :S_SPLIT] = o[:S_SPLIT]
        else:
            out[b, S_SPLIT:] = o[M - (T - S_SPLIT):]
    return out


# revision 26
# speedup vs baseline: 1.2147x; 1.0114x over previous
"""RWKV-style CausalEventModel kernel for 8 Trainium2 NeuronCores.

Strategy (zero cross-core communication):
  - Data-parallel over batch (B=4) x 2-way sequence split per batch = 8 cores.
  - Each core runs the FULL model on M=1088 tokens in channel-major layout
    ([D partitions, tokens free]).  The second-half core starts W=128 tokens
    early with zero initial WKV state; the per-channel decay makes the
    missing-prefix contribution negligible by the output region.
  - Two token half-blocks (512 / 576) are software-pipelined through every
    layer phase; the WKV recurrence state chains across halves via the scan's
    `initial` operand.

V2 performance rework (vs. the first working version):
  - Stats sum-matmuls read the fp32 residual h directly as float32r (1
    cyc/row for N>=256), dropping the bf16 h-copies entirely.
  - LN rows use ACT Rsqrt (one table-set swap per phase into
    reciprocal_sqrt_and_small and back to exp_and_others); everything else
    (Exp/Tanh/Square/Relu/Copy/Identity) lives in exp_and_others.
  - sigmoid(x) = 0.5*(tanh(x/2)+1): computed as Tanh on the ACT engine with
    the 0.5 folded into Wv / Wcv host-side, so no sigmoid table set is needed.
  - WKV assembly fused into scalar_tensor_tensor with e^u as the per-channel
    scalar: num = ekv*e^u + A_shift, den = ek*e^u + B_shift.  1/den uses the
    single-instruction DVE reciprocal_approx_fast (fp32, ~18 bits).
  - Channel-mix relu()^2: ACT Relu epilogue + one in-place 4-wide DVE
    tensor_tensor square per group at bf16 2x rate.
  - Elementwise ops keep bf16 step-1 4B-aligned operands wherever possible so
    DVE runs in its 2x packed mode; z/d bulk ops ride on GpSimd to keep DVE
    free for scans/STT.
"""
import numpy as np
import ml_dtypes

B, T, E, D, F, L, OUT = 4, 2048, 4, 512, 2048, 8, 512
P = 128
KD = D // P          # 4
KF = F // P          # 16
W_WARM = 128
M = (T + W_WARM) // 2        # 1088 tokens per core
S_SPLIT = M                  # first-half output rows
HALVES = [(0, 512), (512, M - 512)]          # token half-blocks per core
N_CORES = 8
EPS = 1e-5

_CACHE = {}


def _mm_slices(cn):
    """Output-column slices (relative to a PSUM tile start) that keep each
    matmul's output inside one 2KB PSUM bank."""
    out = [(0, min(512, cn))]
    if cn > 512:
        out.append((512, cn - 512))
    return out


def _build_bass():
    import concourse.bass as bass
    import concourse.bacc as bacc
    import concourse.mybir as mybir
    import concourse.tile as tile
    from contextlib import ExitStack

    f32 = mybir.dt.float32
    f32r = mybir.dt.float32r
    b16 = mybir.dt.bfloat16
    AF = mybir.ActivationFunctionType
    OP = mybir.AluOpType

    nc = bacc.Bacc("TRN2", target_bir_lowering=False, debug=False)

    # ---------------- DRAM tensors ----------------
    xT_d = nc.dram_tensor("xT", [P, M], b16, kind="ExternalInput")
    wemb_d = nc.dram_tensor("wemb", [P, D], b16, kind="ExternalInput")
    inv_d = nc.dram_tensor("inv", [P, 12], f32, kind="ExternalInput")
    wk_d = nc.dram_tensor("wk", [L, KD, P, D], b16, kind="ExternalInput")
    wv_d = nc.dram_tensor("wv", [L, KD, P, D], b16, kind="ExternalInput")
    wr_d = nc.dram_tensor("wr", [L, KD, P, D], b16, kind="ExternalInput")
    wo_d = nc.dram_tensor("wo", [L, KD, P, D], b16, kind="ExternalInput")
    wck_d = nc.dram_tensor("wck", [L, KD, P, F], b16, kind="ExternalInput")
    wcv_d = nc.dram_tensor("wcv", [L, KF, P, D], b16, kind="ExternalInput")
    wcr_d = nc.dram_tensor("wcr", [L, KD, P, D], b16, kind="ExternalInput")
    whead_d = nc.dram_tensor("whead", [KD, P, OUT], b16, kind="ExternalInput")
    tmv_d = nc.dram_tensor("tmv", [L, P, 32], f32, kind="ExternalInput")
    cmv_d = nc.dram_tensor("cmv", [L, P, 28], f32, kind="ExternalInput")
    headb_d = nc.dram_tensor("headb", [P, KD], f32, kind="ExternalInput")
    out_d = nc.dram_tensor("outT", [KD, P, M], f32, kind="ExternalOutput")

    ctx = ExitStack()
    tc = ctx.enter_context(tile.TileContext(nc))
    sb = ctx.enter_context(tc.tile_pool(name="sb", bufs=1))
    vp = ctx.enter_context(tc.tile_pool(name="vp", bufs=2))
    wp = ctx.enter_context(tc.tile_pool(name="wp", bufs=1))
    pp = ctx.enter_context(tc.tile_pool(name="pp", bufs=2, space="PSUM"))

    # persistent tiles
    h = [sb.tile([P, M], f32, name=f"h{j}", tag=f"h{j}") for j in range(KD)]
    ones_b = sb.tile([P, 1], b16, name="ones_b", tag="ones_b")
    nc.vector.memset(ones_b, 1.0)
    ones_f = sb.tile([P, 1], f32, name="ones_f", tag="ones_f")
    nc.vector.memset(ones_f, 1.0)
    ones_row = sb.tile([1, P], b16, name="ones_row", tag="ones_row")
    nc.vector.memset(ones_row, 1.0)
    srowA = sb.tile([1, M], f32, name="srowA", tag="srowA")
    srowB = sb.tile([1, M], f32, name="srowB", tag="srowB")
    rb0 = sb.tile([1, M], b16, name="rb0", tag="rb0")
    rb1 = sb.tile([1, M], b16, name="rb1", tag="rb1")
    eps_col = sb.tile([P, 1], f32, name="eps_col", tag="eps_col")
    nc.vector.memset(eps_col, EPS)
    rstd_sb = sb.tile([P, M], b16, name="rstd_sb", tag="rstd_sb")
    mean_sb = sb.tile([P, M], b16, name="mean_sb", tag="mean_sb")

    def alloc4(prefix, width, dtype, pool=sb, tagp=None):
        tagp = tagp or prefix
        return [pool.tile([P, width], dtype, name=f"{prefix}{j}", tag=f"{tagp}{j}")
                for j in range(KD)]

    def ln_stats(h_tiles, sq_tiles, z_tiles, c0, cn):
        """Per-token mean/rstd of h[:, c0:c0+cn] over 512 channels into
        rstd_sb / mean_sb (bf16 broadcast tiles, absolute token columns).
        Leaves a bf16 copy of h in z[:, 2+c0:] for the sum matmul; LN-apply
        then normalizes z in place.  z tiles are [P, 2+M] so the partition
        pitch stays 4B-aligned (DVE 2x packed mode eligibility)."""
        cs = slice(c0, c0 + cn)
        zs = slice(2 + c0, 2 + c0 + cn)
        for j in range(KD):
            nc.vector.tensor_copy(out=z_tiles[j][:, zs], in_=h_tiles[j][:, cs])
        # sq = h^2 in bf16 (Square is in every ACT table set)
        for j in range(KD):
            nc.scalar.activation(out=sq_tiles[j][:, cs], in_=h_tiles[j][:, cs],
                                 func=AF.Square)
        for (s0, sn) in _mm_slices(cn):
            a0 = c0 + s0
            sl = slice(a0, a0 + sn)
            sum_ps = pp.tile([1, 512], f32, name="sum_ps", tag="st")
            sq_ps = pp.tile([1, 512], f32, name="sq_ps", tag="st")
            for j in range(KD):
                nc.tensor.matmul(sum_ps[0:1, :sn],
                                 lhsT=ones_b,
                                 rhs=z_tiles[j][:, 2 + a0:2 + a0 + sn],
                                 start=(j == 0), stop=(j == KD - 1))
            for j in range(KD):
                nc.tensor.matmul(sq_ps[0:1, :sn], lhsT=ones_b,
                                 rhs=sq_tiles[j][:, sl],
                                 start=(j == 0), stop=(j == KD - 1))
            # row math on partition 0: var = sqsum/D - (sum/D)^2
            sB = srowB[0:1, sl]
            nc.scalar.activation(out=sB, in_=sum_ps[0:1, :sn], func=AF.Square,
                                 scale=1.0 / D)
            nc.vector.scalar_tensor_tensor(out=srowA[0:1, sl],
                                           in0=sq_ps[0:1, :sn],
                                           scalar=1.0 / D, in1=sB,
                                           op0=OP.mult, op1=OP.subtract)
            nc.scalar.activation(out=srowA[0:1, sl], in_=srowA[0:1, sl],
                                 func=AF.Sqrt, bias=eps_col[0:1, :])
            nc.vector.reciprocal_approx_fast(out=srowB[0:1, sl],
                                             in_=srowA[0:1, sl])
            with nc.allow_low_precision(reason="per-token rstd in bf16"):
                nc.vector.tensor_copy(out=rb0[0:1, sl], in_=srowB[0:1, sl])
                nc.scalar.activation(out=rb1[0:1, sl], in_=sum_ps[0:1, :sn],
                                     func=AF.Copy, scale=1.0 / D)
            # broadcast across partitions: K=1 matmul -> PSUM -> bf16 SBUF
            bc_ps = pp.tile([P, 576], f32, name="bc_ps", tag="mm", bufs=3)
            bc_ps2 = pp.tile([P, 576], f32, name="bc_ps2", tag="mm", bufs=3)
            nc.tensor.matmul(bc_ps[:, :sn], lhsT=ones_row, rhs=rb0[0:1, sl],
                             start=True, stop=True)
            nc.tensor.matmul(bc_ps2[:, :sn], lhsT=ones_row, rhs=rb1[0:1, sl],
                             start=True, stop=True)
            nc.scalar.activation(out=rstd_sb[:, sl], in_=bc_ps[:, :sn],
                                 func=AF.Copy)
            nc.scalar.activation(out=mean_sb[:, sl], in_=bc_ps2[:, :sn],
                                 func=AF.Copy)

    def ka(src_tile, c0):
        """HAM keep-alive: a tiny matmul that depends on a just-produced
        elementwise result, so the PE sees activity inside long vector-only
        windows and its clock stays at K=8/8 (2.4 GHz)."""
        ka_ps = pp.tile([1, 512], f32, name="ka_ps", tag="st")
        nc.tensor.matmul(ka_ps[0:1, :64], lhsT=ones_b,
                         rhs=src_tile[:, c0:c0 + 64],
                         start=True, stop=True)

    def ln_apply_z(z_tiles, c0, cn):
        """z = (z - mean)*rstd in place on the half-block (GpSimd)."""
        cs = slice(c0, c0 + cn)
        zs = slice(2 + c0, 2 + c0 + cn)
        for j in range(KD):
            eng = nc.vector if j % 2 == 0 else nc.gpsimd
            eng.tensor_tensor(z_tiles[j][:, zs], z_tiles[j][:, zs],
                              mean_sb[:, cs], OP.subtract)
        for j in range(KD):
            eng = nc.vector if j % 2 == 0 else nc.gpsimd
            eng.tensor_tensor(z_tiles[j][:, zs], z_tiles[j][:, zs],
                              rstd_sb[:, cs], OP.mult)
            if j % 2 == 1:
                ka(z_tiles[j], 2 + c0)

    def mixes(z_t, d_t, outs_scalars, vec_t, c0, cn, kas=False):
        """out = mix*z + (1-mix)*z_sh = z + (mix-1)*d with d = z - z_sh.
        The host packs (mix-1) so both STT tensor operands read at aligned
        even offsets -> DVE 2x.  d lives in the dead sq tiles."""
        for j in range(KD):
            zc = z_t[j][:, 2 + c0:2 + c0 + cn]
            zsh = z_t[j][:, 1 + c0:1 + c0 + cn]
            eng = nc.vector if j % 2 == 0 else nc.gpsimd
            eng.tensor_tensor(d_t[j][:, c0:c0 + cn], zc, zsh, OP.subtract)
            if kas and j % 2 == 1:
                ka(d_t[j], c0)
        for (out_tiles, col) in outs_scalars:
            for j in range(KD):
                zc = z_t[j][:, 2 + c0:2 + c0 + cn]
                nc.vector.scalar_tensor_tensor(out=out_tiles[j][:, c0:c0 + cn],
                                               in0=d_t[j][:, c0:c0 + cn],
                                               scalar=vec_t[:, col + j:col + j + 1],
                                               in1=zc, op0=OP.mult, op1=OP.add)

    def proj(rhs_tiles, w_t, c0, cn, epilogue):
        """epilogue(m, ps) consumes the [P, cn] PSUM of output tile m."""
        for m in range(KD):
            ps = pp.tile([P, 576], f32, name="proj_ps", tag="mm", bufs=3)
            for (s0, sn) in _mm_slices(cn):
                for kj in range(KD):
                    nc.tensor.matmul(
                        ps[:, s0:s0 + sn],
                        lhsT=w_t[:, kj, m * P:(m + 1) * P],
                        rhs=rhs_tiles[kj][:, c0 + s0:c0 + s0 + sn],
                        start=(kj == 0), stop=(kj == KD - 1))
            epilogue(m, ps)

    # ---------------- embedding ----------------
    xt = sb.tile([P, M], b16, name="xt", tag="xt")
    nc.gpsimd.dma_start(out=xt, in_=xT_d[:, :])
    wemb_t = sb.tile([P, D], b16, name="wemb_t", tag="wemb_t")
    nc.gpsimd.dma_start(out=wemb_t, in_=wemb_d[:, :])
    inv_t = sb.tile([P, 12], f32, name="inv_t", tag="inv_t")
    nc.gpsimd.dma_start(out=inv_t, in_=inv_d[:, :])
    headb_t = sb.tile([P, KD], f32, name="headb_t", tag="headb_t")
    nc.gpsimd.dma_start(out=headb_t, in_=headb_d[:, :])

    for (c0, cn) in HALVES:
        for m in range(KD):
            ps = pp.tile([P, 576], f32, name=f"emb_ps{m}", tag="mm", bufs=3)
            for (s0, sn) in _mm_slices(cn):
                nc.tensor.matmul(ps[:, s0:s0 + sn],
                                 lhsT=wemb_t[:, m * P:(m + 1) * P],
                                 rhs=xt[:, c0 + s0:c0 + s0 + sn],
                                 start=True, stop=True)
            nc.scalar.activation(out=h[m][:, c0:c0 + cn], in_=ps[:, :cn],
                                 func=AF.Identity, bias=inv_t[:, m:m + 1])

    # ln_in (explicit w/b application since h is the residual stream)
    sq = alloc4("sq", M, b16)
    z = [sb.tile([P, 2 + M], b16, name=f"z{j}", tag=f"z{j}") for j in range(KD)]
    for j in range(KD):
        nc.vector.memset(z[j][:, 1:2], 0.0)
    for (c0, cn) in HALVES:
        ln_stats(h, sq, z, c0, cn)
        ln_apply_z(z, c0, cn)
        for j in range(KD):
            nc.vector.tensor_scalar(out=h[j][:, c0:c0 + cn],
                                    in0=z[j][:, 2 + c0:2 + c0 + cn],
                                    scalar1=inv_t[:, 4 + j:5 + j],
                                    scalar2=inv_t[:, 8 + j:9 + j],
                                    op0=OP.mult, op1=OP.add)

    # ---------------- layers ----------------
    for li in range(L):
        tmv_t = vp.tile([P, 32], f32, name=f"tmv{li}", tag="tmv")
        nc.sync.dma_start(out=tmv_t, in_=tmv_d[li])
        wk_t = wp.tile([P, KD, D], b16, name=f"wk{li}", tag="wk", bufs=2)
        nc.sync.dma_start(out=wk_t, in_=wk_d[li].rearrange("k p d -> p k d"))
        wv_t = wp.tile([P, KD, D], b16, name=f"wv{li}", tag="wv", bufs=2)
        nc.sync.dma_start(out=wv_t, in_=wv_d[li].rearrange("k p d -> p k d"))
        wr_t = wp.tile([P, KD, D], b16, name=f"wr{li}", tag="wrc")
        nc.sync.dma_start(out=wr_t, in_=wr_d[li].rearrange("k p d -> p k d"))
        wo_t = wp.tile([P, KD, D], b16, name=f"wo{li}", tag="wock")
        nc.sync.dma_start(out=wo_t, in_=wo_d[li].rearrange("k p d -> p k d"))

        # ---- time mixing ----
        sq = alloc4("sq", M, b16)
        z = [sb.tile([P, 2 + M], b16, name=f"z{j}", tag=f"z{j}")
             for j in range(KD)]
        xkz = alloc4("xkz", M, b16)
        xvz = alloc4("xvz", M, b16)
        xrz = alloc4("xrz", M, b16)
        # ek/vv/rr/sc live in the channel-mix kf slots (disjoint lifetime)
        ek = alloc4("ek", M, b16, tagp="kfA_")
        vv = alloc4("vv", M, b16, tagp="kfB_")
        rr = alloc4("rr", M, b16, tagp="kfC_")
        sc = [sb.tile([P, 2 + M], b16, name=f"sc{j}", tag=f"kfD_{j}")
              for j in range(KD)]
        scB = [sb.tile([P, 2 + M], b16, name=f"scB{j}", tag=f"t2_{j}")
               for j in range(KD)]
        den = [sb.tile([P, 576], f32, name=f"den{j}", tag=f"den{j}")
               for j in range(KD)]
        for j in range(KD):
            nc.vector.memset(z[j][:, 1:2], 0.0)
            nc.vector.memset(sc[j][:, 0:1], 0.0)
            nc.vector.memset(scB[j][:, 0:1], 0.0)

        # stats for both halves first: one Rsqrt table-set period per phase
        for (c0, cn) in HALVES:
            ln_stats(h, sq, z, c0, cn)

        def tm_mix_proj(c0, cn):
            ln_apply_z(z, c0, cn)
            mixes(z, sq, [(xkz, 0), (xvz, 4), (xrz, 8)], tmv_t, c0, cn,
                  kas=True)

            def k_epi(m, ps):
                nc.scalar.activation(out=ek[m][:, c0:c0 + cn], in_=ps[:, :cn],
                                     func=AF.Exp,
                                     bias=tmv_t[:, 20 + m:21 + m])

            def v_epi(m, ps):
                nc.scalar.activation(out=vv[m][:, c0:c0 + cn], in_=ps[:, :cn],
                                     func=AF.Identity,
                                     bias=tmv_t[:, 24 + m:25 + m])

            def r_epi(m, ps):
                nc.scalar.activation(out=rr[m][:, c0:c0 + cn], in_=ps[:, :cn],
                                     func=AF.Tanh, scale=0.5,
                                     bias=tmv_t[:, 28 + m:29 + m])

            proj(xkz, wk_t, c0, cn, k_epi)
            proj(xvz, wv_t, c0, cn, v_epi)
            proj(xrz, wr_t, c0, cn, r_epi)

        def tm_wkv(c0, cn):
            cs = slice(c0, c0 + cn)
            for j in range(KD):
                # ekv into the xvz slot (v-proj has fully consumed it)
                nc.vector.tensor_tensor(xvz[j][:, cs], ek[j][:, cs],
                                        vv[j][:, cs], OP.mult)
            for j in range(KD):
                ew_ap = tmv_t[:, 12 + j:13 + j].to_broadcast([P, cn])
                init = 0.0 if c0 == 0 else sc[j][:, c0:c0 + 1]
                nc.vector.tensor_tensor_scan(out=sc[j][:, 1 + c0:1 + c0 + cn],
                                             data0=ew_ap, data1=xvz[j][:, cs],
                                             initial=init,
                                             op0=OP.mult, op1=OP.add)
                ka(sc[j], c0)
            for j in range(KD):
                ew_ap = tmv_t[:, 12 + j:13 + j].to_broadcast([P, cn])
                init = 0.0 if c0 == 0 else scB[j][:, c0:c0 + 1]
                nc.vector.tensor_tensor_scan(out=scB[j][:, 1 + c0:1 + c0 + cn],
                                             data0=ew_ap, data1=ek[j][:, cs],
                                             initial=init,
                                             op0=OP.mult, op1=OP.add)
                if j % 2 == 1:
                    ka(scB[j], c0)
            for j in range(KD):
                # num = ekv*e^u + A_shift  (overwrite vv; all operands at
                # aligned even offsets -> DVE 2x)
                nc.vector.scalar_tensor_tensor(out=vv[j][:, cs],
                                               in0=xvz[j][:, cs],
                                               scalar=tmv_t[:, 16 + j:17 + j],
                                               in1=sc[j][:, c0:c0 + cn],
                                               op0=OP.mult, op1=OP.add)
            for j in range(KD):
                # den = ek*e^u + B_shift (fp32 for the fast reciprocal)
                nc.vector.scalar_tensor_tensor(out=den[j][:, :cn],
                                               in0=ek[j][:, cs],
                                               scalar=tmv_t[:, 16 + j:17 + j],
                                               in1=scB[j][:, c0:c0 + cn],
                                               op0=OP.mult, op1=OP.add)
            for j in range(KD):
                nc.vector.reciprocal_approx_fast(out=den[j][:, :cn],
                                                 in_=den[j][:, :cn])
            for j in range(KD):
                # rn = (tanh+1)*recip  (0.5 folded into Wv host-side)
                nc.vector.scalar_tensor_tensor(out=rr[j][:, cs],
                                               in0=rr[j][:, cs], scalar=1.0,
                                               in1=den[j][:, :cn],
                                               op0=OP.add, op1=OP.mult)
            for j in range(KD):
                # rwkv = num * (rr_scaled * recip) -> k-proj's dead xkz slot
                nc.vector.tensor_tensor(xkz[j][:, cs], vv[j][:, cs],
                                        rr[j][:, cs], OP.mult)

        def tm_out(c0, cn):
            cs = slice(c0, c0 + cn)

            def o_epi(m, ps):
                nc.vector.tensor_tensor(h[m][:, cs], h[m][:, cs], ps[:, :cn],
                                        OP.add)

            proj(xkz, wo_t, c0, cn, o_epi)

        tm_mix_proj(*HALVES[0])
        tm_mix_proj(*HALVES[1])
        tm_wkv(*HALVES[0])
        tm_out(*HALVES[0])
        tm_wkv(*HALVES[1])
        tm_out(*HALVES[1])

        # ================= channel mixing =================
        cmv_t = vp.tile([P, 28], f32, name=f"cmv{li}", tag="cmv")
        nc.sync.dma_start(out=cmv_t, in_=cmv_d[li])
        wck_t = wp.tile([P, KD, F], b16, name=f"wck{li}", tag="wock")
        nc.sync.dma_start(out=wck_t, in_=wck_d[li].rearrange("k p d -> p k d"))
        wcv_t = wp.tile([P, KF, D], b16, name=f"wcv{li}", tag="wcv")
        nc.sync.dma_start(out=wcv_t, in_=wcv_d[li].rearrange("k p d -> p k d"))
        wcr_t = wp.tile([P, KD, D], b16, name=f"wcr{li}", tag="wrc")
        nc.sync.dma_start(out=wcr_t, in_=wcr_d[li].rearrange("k p d -> p k d"))

        sq = alloc4("sq", M, b16)
        z = [sb.tile([P, 2 + M], b16, name=f"z{j}", tag=f"z{j}")
             for j in range(KD)]
        xkc = alloc4("xkc", M, b16, tagp="xkz")
        xrc = alloc4("xrc", M, b16, tagp="xrz")
        rf = alloc4("rf", M, b16, tagp="xvz")
        kf_t = [sb.tile([P, 4, 576], b16, name=f"kft{g}", tag=f"kf{ch}_{g % KD}")
                for g, ch in zip(range(4), "ABCD")]
        for j in range(KD):
            nc.vector.memset(z[j][:, 1:2], 0.0)

        for (c0, cn) in HALVES:
            ln_stats(h, sq, z, c0, cn)

        def cm_half(c0, cn):
            cs = slice(c0, c0 + cn)
            ln_apply_z(z, c0, cn)
            mixes(z, sq, [(xkc, 0), (xrc, 4)], cmv_t, c0, cn, kas=True)

            def rf_epi(m, ps):
                nc.scalar.activation(out=rf[m][:, c0:c0 + cn], in_=ps[:, :cn],
                                     func=AF.Tanh, scale=0.5,
                                     bias=cmv_t[:, 8 + m:9 + m])

            proj(xrc, wcr_t, c0, cn, rf_epi)
            for g in range(4):
                for jj in range(4):
                    fo = g * 4 + jj
                    kfp = pp.tile([P, 576], f32, name="kfp", bufs=3,
                                  tag="mm")
                    for (s0, sn) in _mm_slices(cn):
                        for kj in range(KD):
                            nc.tensor.matmul(
                                kfp[:, s0:s0 + sn],
                                lhsT=wck_t[:, kj, fo * P:(fo + 1) * P],
                                rhs=xkc[kj][:, c0 + s0:c0 + s0 + sn],
                                start=(kj == 0), stop=(kj == KD - 1))
                    nc.scalar.activation(out=kf_t[g][:, jj, :cn], in_=kfp[:, :cn],
                                         func=AF.Relu,
                                         bias=cmv_t[:, 12 + fo:13 + fo])
                nc.vector.tensor_tensor(kf_t[g][:, :, :cn], kf_t[g][:, :, :cn],
                                        kf_t[g][:, :, :cn], OP.mult)
            for m in range(KD):
                wvps = pp.tile([P, 576], f32, name="wvps", tag="mm", bufs=3)
                for (s0, sn) in _mm_slices(cn):
                    for kj in range(KF):
                        g, jj = kj // 4, kj % 4
                        nc.tensor.matmul(wvps[:, s0:s0 + sn],
                                         lhsT=wcv_t[:, kj, m * P:(m + 1) * P],
                                         rhs=kf_t[g][:, jj, s0:s0 + sn],
                                         start=(kj == 0), stop=(kj == KF - 1))
                t2 = vp.tile([P, 576], f32, name="t2", tag="t2s")
                # h += (tanh+1) * wvps  (0.5 folded into Wcv host-side)
                nc.vector.scalar_tensor_tensor(out=t2[:, :cn],
                                               in0=rf[m][:, cs], scalar=1.0,
                                               in1=wvps[:, :cn],
                                               op0=OP.add, op1=OP.mult)
                nc.gpsimd.dma_start(out=h[m][:, cs], in_=t2[:, :cn],
                                    accum_op=OP.add)

        cm_half(*HALVES[0])
        cm_half(*HALVES[1])

    # ---------------- final LN + head ----------------
    sq = alloc4("sq", M, b16)
    zf = [sb.tile([P, 2 + M], b16, name=f"z{j}", tag=f"z{j}") for j in range(KD)]
    whead_t = wp.tile([P, KD, OUT], b16, name="whead_t", tag="wock")
    nc.sync.dma_start(out=whead_t, in_=whead_d.rearrange("k p d -> p k d"))
    for (c0, cn) in HALVES:
        ln_stats(h, sq, zf, c0, cn)
        ln_apply_z(zf, c0, cn)
        for m in range(KD):
            ps = pp.tile([P, 576], f32, name="head_ps", tag="mm", bufs=3)
            for (s0, sn) in _mm_slices(cn):
                for kj in range(KD):
                    nc.tensor.matmul(
                        ps[:, s0:s0 + sn],
                        lhsT=whead_t[:, kj, m * P:(m + 1) * P],
                        rhs=zf[kj][:, 2 + c0 + s0:2 + c0 + s0 + sn],
                        start=(kj == 0), stop=(kj == KD - 1))
            nc.scalar.activation(out=h[m][:, c0:c0 + cn], in_=ps[:, :cn],
                                 func=AF.Identity, bias=headb_t[:, m:m + 1])
            nc.sync.dma_start(out=out_d[m][:, c0:c0 + cn],
                              in_=h[m][:, c0:c0 + cn])

    ctx.close()
    nc.compile()
    return nc


def _pack_cols(vec, kd=KD):
    """[kd*P] -> [P, kd] so that column j holds channels j*P..(j+1)*P-1."""
    return np.ascontiguousarray(vec.reshape(kd, P).T)


def _prep_inputs(inputs):
    bf16 = ml_dtypes.bfloat16
    f32 = np.float32
    inp = {k: np.asarray(v, dtype=f32) for k, v in inputs.items()}

    shared = {}
    wemb_p = np.zeros((P, D), f32)
    wemb_p[:E] = inp["emb_w"]
    shared["wemb"] = wemb_p.astype(bf16)
    shared["inv"] = np.concatenate(
        [_pack_cols(inp["emb_b"]), _pack_cols(inp["ln_in_w"]),
         _pack_cols(inp["ln_in_b"])], axis=1).astype(f32)

    def fold(w_vec, mat):
        return (w_vec[:, None] * mat)

    wk = np.stack([fold(inp["ln0_w"][i], inp["tm_wk"][i]) for i in range(L)])
    # 0.5 folded into Wv: sigmoid(r) = 0.5*(tanh(r/2)+1), the 0.5 rides on v
    wv = np.stack([0.5 * fold(inp["ln0_w"][i], inp["tm_wv"][i])
                   for i in range(L)])
    wr = np.stack([fold(inp["ln0_w"][i], inp["tm_wr"][i]) for i in range(L)])
    wo = inp["tm_wo"]
    wck = np.stack([fold(inp["ln1_w"][i], inp["cm_wk"][i]) for i in range(L)])
    wcr = np.stack([fold(inp["ln1_w"][i], inp["cm_wr"][i]) for i in range(L)])
    wcv = 0.5 * inp["cm_wv"]

    shared["wk"] = wk.reshape(L, KD, P, D).astype(bf16)
    shared["wv"] = wv.reshape(L, KD, P, D).astype(bf16)
    shared["wr"] = wr.reshape(L, KD, P, D).astype(bf16)
    shared["wo"] = wo.reshape(L, KD, P, D).astype(bf16)
    shared["wck"] = wck.reshape(L, KD, P, F).astype(bf16)
    shared["wcv"] = wcv.reshape(L, KF, P, D).astype(bf16)
    shared["wcr"] = wcr.reshape(L, KD, P, D).astype(bf16)
    shared["whead"] = (inp["ln_out_w"][:, None] * inp["head_w"]).reshape(
        KD, P, OUT).astype(bf16)
    shared["headb"] = _pack_cols(inp["ln_out_b"] @ inp["head_w"]).astype(f32)

    tmv = np.zeros((L, P, 32), f32)
    cmv = np.zeros((L, P, 28), f32)
    for i in range(L):
        ew = np.exp(-np.exp(inp["tm_decay"][i]))
        tmv[i, :, 0:4] = _pack_cols(inp["tm_mix_k"][i] - 1.0)
        tmv[i, :, 4:8] = _pack_cols(inp["tm_mix_v"][i] - 1.0)
        tmv[i, :, 8:12] = _pack_cols(inp["tm_mix_r"][i] - 1.0)
        tmv[i, :, 12:16] = _pack_cols(ew)
        tmv[i, :, 16:20] = _pack_cols(np.exp(inp["tm_first"][i]))
        tmv[i, :, 20:24] = _pack_cols(inp["ln0_b"][i] @ inp["tm_wk"][i])
        tmv[i, :, 24:28] = _pack_cols(0.5 * (inp["ln0_b"][i] @ inp["tm_wv"][i]))
        tmv[i, :, 28:32] = _pack_cols(0.5 * (inp["ln0_b"][i] @ inp["tm_wr"][i]))
        cmv[i, :, 0:4] = _pack_cols(inp["cm_mix_k"][i] - 1.0)
        cmv[i, :, 4:8] = _pack_cols(inp["cm_mix_r"][i] - 1.0)
        cmv[i, :, 8:12] = _pack_cols(0.5 * (inp["ln1_b"][i] @ inp["cm_wr"][i]))
        cmv[i, :, 12:28] = _pack_cols(inp["ln1_b"][i] @ inp["cm_wk"][i], kd=KF)
    shared["tmv"] = tmv
    shared["cmv"] = cmv

    in_maps = []
    x = inp["x"]
    for c in range(N_CORES):
        b, half = c // 2, c % 2
        t0 = 0 if half == 0 else T - M
        x_sl = np.zeros((P, M), f32)
        x_sl[:E] = x[b, t0:t0 + M].T
        m = dict(shared)
        m["xT"] = x_sl.astype(bf16)
        in_maps.append(m)
    return in_maps


TRACE = False  # set by test harness to capture an NTFF profile


def kernel(**inputs):
    from concourse import bass_utils

    if "nc" not in _CACHE:
        _CACHE["nc"] = _build_bass()
    nc = _CACHE["nc"]
    in_maps = _prep_inputs(inputs)
    res = bass_utils.run_bass_kernel_spmd(nc, in_maps, core_ids=list(range(N_CORES)),
                                          trace=TRACE)
    _CACHE["last_res"] = res
    out = np.zeros((B, T, OUT), np.float32)
    for c in range(N_CORES):
        b, half = c // 2, c % 2
        oT = res.results[c]["outT"].reshape(D, M)  # [channels, tokens]
        o = np.ascontiguousarray(oT.T)             # [tokens, channels]
        if half == 0:
            out[b, :S_SPLIT] = o[:S_SPLIT]
        else:
            out[b, S_SPLIT:] = o[M - (T - S_SPLIT):]
    return out
